# revision 15
# baseline (speedup 1.0000x reference)
"""FPLPGCN (2x GCNConv feature prop + 10x label prop + fuse) on 8 trn2 cores.

Strategy (graph/data parallel, per sharding hint):
- Nodes are globally sorted by in-degree and striped round-robin across the 8
  cores (degree-matched windows).  Each core owns NPAD=12544 node rows (44
  zero "fake" rows pad 100000 -> 100352).
- GCN math is refactored so aggregation is a pure gather+sum:
      out[n] = dinv[n] * (sum_{e->n} u'[src_e] + u'[n]) + b,
  with u' = dinv * (z @ W) the "table" that is AllGather'ed each round.
- Per conv round: each core computes u' for its nodes (PE matmul), AllGathers
  the table to HBM, then aggregates its in-edges with bulk dma_gather calls
  (node-aligned slot grids, int16 indices, 4 base-offset chunks to cover the
  100k-row table) and DVE strided reduces.
"""

import sys

sys.path.insert(0, "/opt/trn_rl_repo")

import numpy as np

NC = 8
P = 128
NPAD = 12544          # local rows per core (98 windows of 128)
NWIN = NPAD // P      # 98
TABROWS = NC * NPAD   # 100352
CH = TABROWS // 4     # 25088 chunk stride
WINR = 32768          # rows reachable per gather call (int16 >= 0)
SPAN = 3              # windows per gather-call group
IN_DIM, HID, OUT, DW = 128, 64, 32, 64
NUM_LBL = 10
FSLOT = 64            # gather element = 64 f32 = 256B (labels pad 32->64)


# ----------------------------------------------------------------------------
# host-side index preprocessing (pure index manipulation; no FP math on data)
# ----------------------------------------------------------------------------

def _preprocess(edge_index, n_nodes):
    src = np.asarray(edge_index[0], dtype=np.int64)
    dst = np.asarray(edge_index[1], dtype=np.int64)
    E = src.shape[0]
    deg = np.bincount(dst, minlength=n_nodes).astype(np.int64)

    order = np.argsort(deg, kind="stable")          # ascending in-degree
    rank = np.empty(n_nodes, np.int64)
    rank[order] = np.arange(TABROWS - n_nodes, TABROWS)  # fakes get ranks 0..351
    core_of = rank % NC
    local_of = rank // NC
    trow = core_of * NPAD + local_of                # table row per real node

    # per-edge placement
    ecore = core_of[dst]
    eloc = local_of[dst]
    ewin = eloc // P
    epart = eloc % P
    etr_s = trow[src]
    home = etr_s // CH
    flex = ((etr_s % CH) < (WINR - CH)) & (home >= 1)

    # node-chunk slot counts with greedy flex balancing (chunk h -> h-1)
    nodekey = ecore * NPAD + eloc                   # 0..TABROWS-1
    cnt = np.zeros((TABROWS, 4), np.int32)
    np.add.at(cnt, (nodekey[~flex], home[~flex]), 1)
    echunk = home.copy()
    for h in (1, 2, 3):
        m = flex & (home == h)
        if not m.any():
            continue
        f = np.bincount(nodekey[m], minlength=TABROWS)
        lo, hi = cnt[:, h - 1], cnt[:, h]
        x = np.clip((hi + f - lo + 1) // 2, 0, f)   # how many go to lower chunk
        # choose the first x flex edges of each node (stable order)
        o = np.argsort(nodekey[m], kind="stable")
        idxs = np.flatnonzero(m)[o]
        nk = nodekey[idxs]
        starts = np.searchsorted(nk, np.arange(TABROWS))
        pos = np.arange(idxs.size) - starts[nk]
        tolow = pos < x[nk]
        echunk[idxs[tolow]] = h - 1
        np.add.at(cnt, (nk, np.where(tolow, h - 1, h)), 1)

    # common column counts per (window, chunk) across all cores
    K = cnt.reshape(NC, NWIN, P, 4).max(axis=(0, 2))          # [NWIN, 4]

    # k-index of each edge within its (node, chunk) group
    o = np.lexsort((echunk, nodekey))
    nk = nodekey[o]
    ck = echunk[o]
    key = nk * 4 + ck
    first = np.searchsorted(key, key, side="left")
    kidx = np.empty(E, np.int64)
    kidx[o] = np.arange(E) - first

    # span/call layout: spans of SPAN windows; call = (span, chunk)
    nspan = (NWIN + SPAN - 1) // SPAN
    span_of_w = np.arange(NWIN) // SPAN
    # column offset of (w, c) inside call (span(w), c)
    colbase = np.zeros((NWIN, 4), np.int64)
    for s in range(nspan):
        ws = np.arange(s * SPAN, min((s + 1) * SPAN, NWIN))
        for c in range(4):
            acc = 0
            for w in ws:
                colbase[w, c] = acc
                acc += K[w, c]
    call_cols = np.zeros((nspan, 4), np.int64)
    for s in range(nspan):
        ws = slice(s * SPAN, min((s + 1) * SPAN, NWIN))
        call_cols[s] = K[ws].sum(axis=0)
    call_n = call_cols * P                                    # num_idxs per call
    # idx dram layout: calls concatenated (span-major, chunk-minor), each call
    # is n/16 int16 columns wrapped into 16 partitions, replicated x8.
    call_off16 = np.zeros((nspan, 4), np.int64)
    off = 0
    for s in range(nspan):
        for c in range(4):
            call_off16[s, c] = off
            off += call_n[s, c] // 16
    tot16 = off

    # fill idx buffers per core (default 0 -> chunk base row = fake zero row)
    idx_all = np.zeros((NC, 16, tot16), np.int16)
    ci = call_off16[span_of_w[ewin], echunk]                  # per-edge call off
    col = colbase[ewin, echunk] + kidx
    i_in_call = col * P + epart
    rel = etr_s - echunk * CH
    assert rel.min() >= 0 and rel.max() < WINR
    flat = ci * 16 + (i_in_call // 16) * 16 + (i_in_call % 16)
    # layout: [16, tot16]: position i of call -> [i%16, off16 + i//16]
    r16 = i_in_call % 16
    c16 = ci + i_in_call // 16
    idx_all[ecore, r16, c16] = rel.astype(np.int16)

    meta = dict(order=order, rank=rank, core_of=core_of, local_of=local_of,
                trow=trow, deg=deg, K=K, call_cols=call_cols, call_n=call_n,
                call_off16=call_off16, tot16=tot16, nspan=nspan)
    return idx_all, meta


def _shard_nodes(arr, core_of, local_of, width, dtype=np.float32):
    """Scatter full [N, width] node array into per-core [NPAD, width] shards."""
    n = arr.shape[0]
    out = np.zeros((NC, NPAD, width), dtype)
    a2 = np.asarray(arr, dtype).reshape(n, width)
    out[core_of, local_of] = a2
    return out


# ----------------------------------------------------------------------------
# device program
# ----------------------------------------------------------------------------

def _build(meta, nonzero_b):
    import concourse.bacc as bacc
    import concourse.bass as bass
    import concourse.mybir as mybir
    import concourse.tile as tile

    f32 = mybir.dt.float32
    K = meta["K"]
    call_n = meta["call_n"]
    call_off16 = meta["call_off16"]
    tot16 = meta["tot16"]
    nspan = meta["nspan"]
    TAB_ALLOC = 3 * CH + WINR  # rows needed so table[3*CH : 3*CH+WINR] is valid

    nc = bacc.Bacc("TRN2", target_bir_lowering=False, debug=False,
                   num_devices=NC)

    x_sh = nc.dram_tensor("x_sh", [NPAD, IN_DIM], f32, kind="ExternalInput")
    y_sh = nc.dram_tensor("y_sh", [NPAD, OUT], f32, kind="ExternalInput")
    dw_sh = nc.dram_tensor("dw_sh", [NPAD, DW], f32, kind="ExternalInput")
    mask_sh = nc.dram_tensor("mask_sh", [NPAD, 1], mybir.dt.int8,
                             kind="ExternalInput")
    deg_sh = nc.dram_tensor("deg_sh", [NPAD, 1], mybir.dt.int32,
                            kind="ExternalInput")
    idx_d = nc.dram_tensor("idx_d", [P, tot16], mybir.dt.int16,
                           kind="ExternalInput")
    W0_d = nc.dram_tensor("W0", [IN_DIM, HID], f32, kind="ExternalInput")
    W1_d = nc.dram_tensor("W1", [HID, HID], f32, kind="ExternalInput")
    Wl_d = nc.dram_tensor("Wl", [NUM_LBL * OUT, OUT], f32, kind="ExternalInput")
    Wf_d = nc.dram_tensor("Wf", [HID + OUT + DW, OUT], f32, kind="ExternalInput")
    b_d = nc.dram_tensor("b_all", [4, max(HID, OUT) * NUM_LBL], f32,
                         kind="ExternalInput")  # b0|b1|bl(10x32)|bf rows
    out_sh = nc.dram_tensor("out_sh", [NPAD, OUT], f32, kind="ExternalOutput")

    # internal DRAM
    tabF = [nc.dram_tensor(f"tabF{i}", [TAB_ALLOC, FSLOT], f32,
                           addr_space="Shared") for i in range(2)]
    tabL = [nc.dram_tensor(f"tabL{i}", [TAB_ALLOC, FSLOT], f32,
                           addr_space="Shared") for i in range(2)]
    bnF = [nc.dram_tensor(f"bnF{i}", [NPAD, FSLOT], f32) for i in range(2)]
    bnL = [nc.dram_tensor(f"bnL{i}", [NPAD, FSLOT], f32) for i in range(2)]

    with tile.TileContext(nc) as tc:
        with tc.tile_pool(name="persist", bufs=1) as pp, \
             tc.tile_pool(name="g", bufs=2) as gp, \
             tc.tile_pool(name="ix", bufs=2) as ixp, \
             tc.tile_pool(name="wk", bufs=3) as wk, \
             tc.tile_pool(name="ps", bufs=2, space="PSUM") as ps, \
             tc.tile_pool(name="ub", bufs=2) as ub:

            # ---- constants / persistent state ----
            W0 = pp.tile([IN_DIM, HID], f32); nc.sync.dma_start(out=W0[:], in_=W0_d[:, :])
            # W1 / Wl replicated along partition offsets so batched-transpose
            # lhsT slices (base partition 64a / 32a) see rhs at the same base.
            W1 = pp.tile([P, HID], f32)
            for a in range(P // HID):
                nc.sync.dma_start(out=W1[a * HID:(a + 1) * HID, :], in_=W1_d[:, :])
            Wl = pp.tile([P, NUM_LBL * OUT], f32)
            for j in range(NUM_LBL):
                for a in range(P // OUT):
                    nc.sync.dma_start(
                        out=Wl[a * OUT:(a + 1) * OUT, j * OUT:(j + 1) * OUT],
                        in_=Wl_d[j * OUT:(j + 1) * OUT, :])
            Wfa = pp.tile([128, OUT], f32); nc.sync.dma_start(out=Wfa[:], in_=Wf_d[0:128, :])
            Wfb = pp.tile([HID + OUT + DW - 128, OUT], f32)
            nc.sync.dma_start(out=Wfb[:], in_=Wf_d[128:, :])
            from concourse.masks import make_identity
            ident = pp.tile([P, P], f32)
            make_identity(nc, ident[:])

            yb = pp.tile([P, NWIN * OUT], f32)
            nc.sync.dma_start(
                out=yb[:].rearrange("p (w f) -> p w f", w=NWIN),
                in_=y_sh[:, :].rearrange("(w p) f -> p w f", p=P))
            maskb = pp.tile([P, NWIN], mybir.dt.int8)
            nc.sync.dma_start(
                out=maskb[:],
                in_=mask_sh[:, 0].rearrange("(w p) -> p w", p=P))
            degb = pp.tile([P, NWIN], mybir.dt.int32)
            nc.sync.dma_start(
                out=degb[:],
                in_=deg_sh[:, 0].rearrange("(w p) -> p w", p=P))

            degf = pp.tile([P, NWIN], f32)
            nc.vector.tensor_copy(out=degf[:], in_=degb[:])
            recipb = pp.tile([P, NWIN], f32)
            nc.vector.tensor_scalar(out=degf[:], in0=degf[:], scalar1=1.0,
                                    scalar2=None, op0=mybir.AluOpType.add)
            nc.vector.reciprocal(out=recipb[:], in_=degf[:])      # 1/(deg+1)
            dinvb = pp.tile([P, NWIN], f32)
            nc.scalar.sqrt(out=dinvb[:], in_=recipb[:])           # 1/sqrt(deg+1)
            # zero fake lanes (window 0, partitions 0..43)
            nc.vector.memset(recipb[0:44, 0:1], 0.0)
            nc.vector.memset(dinvb[0:44, 0:1], 0.0)
            dinvy = pp.tile([P, NWIN * OUT], f32)
            for w in range(NWIN):
                nc.vector.tensor_scalar(
                    out=dinvy[:, w * OUT:(w + 1) * OUT],
                    in0=yb[:, w * OUT:(w + 1) * OUT],
                    scalar1=dinvb[:, w:w + 1], scalar2=None,
                    op0=mybir.AluOpType.mult)

            # bias broadcast tiles (built only when biases are nonzero)
            def bias_tile(row, width):
                bt = pp.tile([P, width], f32, tag=f"bias{row}_{width}", name=f"bias{row}_{width}")
                onecol = pp.tile([1, P], f32, tag="onecol", name="onecol")
                nc.vector.memset(onecol[:], 1.0)
                brow = pp.tile([1, width], f32, tag=f"brow{row}_{width}", name=f"brow{row}_{width}")
                nc.sync.dma_start(out=brow[:], in_=b_d[row:row + 1, 0:width])
                pt = ps.tile([P, width], f32, tag="biasps", name="biasps")
                nc.tensor.matmul(out=pt[:], lhsT=onecol[:], rhs=brow[:],
                                 start=True, stop=True)
                nc.vector.tensor_copy(out=bt[:], in_=pt[:])
                return bt

            bias0 = bias_tile(0, HID) if nonzero_b[0] else None
            bias1 = bias_tile(1, HID) if nonzero_b[1] else None
            biasf = bias_tile(3, OUT) if nonzero_b[3] else None
            # label biases: one [P, OUT] tile per label conv if any nonzero
            biasl = [None] * NUM_LBL

            vF = pp.tile([P, NWIN * HID], f32)      # current v (feature chain)
            vL = pp.tile([P, NWIN * OUT], f32)      # current v (label chain)
            hfin = pp.tile([P, NWIN * HID], f32)    # final h (node major)
            xlfin = pp.tile([P, NWIN * OUT], f32)   # final xl (node major)

            # ---- helpers ----
            def stage_matmul(vtile, F_in, W_ap, F_out, bounce):
                """u' = v @ W per window -> bounce DRAM."""
                per = min(P // F_in, 3)   # lhsT base partition must be 0/32/64
                for w0 in range(0, NWIN, per):
                    nwt = min(per, NWIN - w0)
                    tp = ps.tile([P, P], f32, tag="tps")
                    nc.tensor.transpose(
                        out=tp[0:nwt * F_in, :],
                        in_=vtile[:, w0 * F_in:(w0 + nwt) * F_in],
                        identity=ident[:])
                    vT = wk.tile([P, P], f32, tag="vT")
                    nc.scalar.copy(out=vT[0:nwt * F_in, :], in_=tp[0:nwt * F_in, :])
                    for a in range(nwt):
                        w = w0 + a
                        up = ps.tile([P, F_out], f32, tag="ups")
                        nc.tensor.matmul(out=up[:],
                                         lhsT=vT[a * F_in:(a + 1) * F_in, :],
                                         rhs=W_ap[a * F_in:(a + 1) * F_in, :],
                                         start=True, stop=True)
                        ut = wk.tile([P, F_out], f32, tag="ut")
                        nc.scalar.copy(out=ut[:], in_=up[:])
                        nc.sync.dma_start(
                            out=bounce[w * P:(w + 1) * P, 0:F_out], in_=ut[:])

            def stage_agg(tab, F_out, bounce, out_cb):
                """Aggregate: out_cb(w, acc_tile) for each window."""
                for s in range(nspan):
                    w_lo = s * SPAN
                    w_hi = min(w_lo + SPAN, NWIN)
                    ncol16 = int(sum(call_n[s, c] // 16 for c in range(4)))
                    if ncol16 == 0:
                        continue
                    ixt = ixp.tile([P, ncol16], mybir.dt.int16, tag="ix")
                    base16 = int(call_off16[s, 0])
                    nc.sync.dma_start(out=ixt[:],
                                      in_=idx_d[:, base16:base16 + ncol16])
                    span_cols = int(sum(K[w_lo:w_hi, c].sum() for c in range(4)))
                    g = gp.tile([P, span_cols * FSLOT], f32, tag="g")
                    coff = 0
                    reg = {}
                    SUBCOLS = 8       # <=1024 idxs per dma_gather call
                    for c in range(4):
                        n = int(call_n[s, c])
                        if n == 0:
                            continue
                        o16 = int(call_off16[s, c]) - base16
                        ncols = n // P
                        for c0 in range(0, ncols, SUBCOLS):
                            c1 = min(c0 + SUBCOLS, ncols)
                            nsub = (c1 - c0) * P
                            nc.gpsimd.dma_gather(
                                out_ap=g[:, (coff + c0) * FSLOT:(coff + c1) * FSLOT]
                                    .rearrange("p (s f) -> p s f", f=FSLOT),
                                in_ap=tab[c * CH:c * CH + WINR, :],
                                idxs_ap=ixt[:, o16 + c0 * 8:o16 + c1 * 8],
                                num_idxs=nsub, num_idxs_reg=nsub,
                                elem_size=FSLOT)
                        # record region columns per window
                        cc = coff
                        for w in range(w_lo, w_hi):
                            if K[w, c]:
                                reg.setdefault(w, []).append((cc, int(K[w, c])))
                                cc += int(K[w, c])
                        coff += n // P
                    for w in range(w_lo, w_hi):
                        acc = wk.tile([P, F_out], f32, tag="acc")
                        selft = wk.tile([P, F_out], f32, tag="selft")
                        nc.sync.dma_start(out=selft[:],
                                          in_=bounce[w * P:(w + 1) * P, 0:F_out])
                        cur = selft[:]
                        first = True
                        for (cstart, ncols) in reg.get(w, []):
                            gv = g[:, cstart * FSLOT:(cstart + ncols) * FSLOT] \
                                .rearrange("p (k f) -> p f k", f=FSLOT)
                            if first:
                                nc.vector.reduce_sum(
                                    out=acc[:], in_=gv[:, 0:F_out, :],
                                    axis=mybir.AxisListType.X)
                                first = False
                            else:
                                t = wk.tile([P, F_out], f32, tag="rt")
                                nc.vector.reduce_sum(
                                    out=t[:], in_=gv[:, 0:F_out, :],
                                    axis=mybir.AxisListType.X)
                                nc.vector.tensor_add(out=acc[:], in0=acc[:],
                                                     in1=t[:])
                        if first:
                            nc.vector.tensor_copy(out=acc[:], in_=cur)
                        else:
                            nc.vector.tensor_add(out=acc[:], in0=acc[:],
                                                 in1=cur)
                        out_cb(w, acc)

            # ---- feature conv 1: v_x = dinv*x ; u_f1 = v_x @ W0 ----
            for w in range(NWIN):
                xt = wk.tile([P, IN_DIM], f32, tag="xt")
                nc.sync.dma_start(out=xt[:], in_=x_sh[w * P:(w + 1) * P, :])
                nc.vector.tensor_scalar(out=xt[:], in0=xt[:],
                                        scalar1=dinvb[:, w:w + 1],
                                        scalar2=None, op0=mybir.AluOpType.mult)
                # transpose+matmul inline (F_in=128: one window per transpose)
                tp = ps.tile([P, P], f32, tag="tps")
                nc.tensor.transpose(out=tp[:], in_=xt[:], identity=ident[:])
                vT = wk.tile([P, P], f32, tag="vT")
                nc.scalar.copy(out=vT[:], in_=tp[:])
                up = ps.tile([P, HID], f32, tag="ups")
                nc.tensor.matmul(out=up[:], lhsT=vT[:], rhs=W0[:], start=True,
                                 stop=True)
                ut = wk.tile([P, HID], f32, tag="ut")
                nc.scalar.copy(out=ut[:], in_=up[:])
                nc.sync.dma_start(out=bnF[0][w * P:(w + 1) * P, 0:HID], in_=ut[:])
            nc.gpsimd.collective_compute(
                "AllGather", bass.mybir.AluOpType.bypass,
                replica_groups=[list(range(NC))],
                ins=[bnF[0][:, :].opt()], outs=[tabF[0][0:TABROWS, :].opt()])

            # ---- label conv 1 input: u_l1 = dinvy @ Wl0 ----
            stage_matmul(dinvy, OUT, Wl[:, 0:OUT], OUT, bnL[0])
            nc.gpsimd.collective_compute(
                "AllGather", bass.mybir.AluOpType.bypass,
                replica_groups=[list(range(NC))],
                ins=[bnL[0][:, :].opt()], outs=[tabL[0][0:TABROWS, :].opt()])

            # ---- feature conv 1 aggregate -> v_f1 ; u_f2 ; AG ----
            def cb_f1(w, acc):
                nc.vector.tensor_scalar(out=vF[:, w * HID:(w + 1) * HID],
                                        in0=acc[:], scalar1=recipb[:, w:w + 1],
                                        scalar2=None, op0=mybir.AluOpType.mult)
                if bias0 is not None:
                    dv = wk.tile([P, HID], f32, tag="dbv")
                    nc.vector.tensor_scalar(out=dv[:], in0=bias0[:],
                                            scalar1=dinvb[:, w:w + 1],
                                            scalar2=None,
                                            op0=mybir.AluOpType.mult)
                    nc.vector.tensor_add(out=vF[:, w * HID:(w + 1) * HID],
                                         in0=vF[:, w * HID:(w + 1) * HID],
                                         in1=dv[:])
            stage_agg(tabF[0], HID, bnF[0], cb_f1)
            stage_matmul(vF, HID, W1[:, :], HID, bnF[1])
            nc.gpsimd.collective_compute(
                "AllGather", bass.mybir.AluOpType.bypass,
                replica_groups=[list(range(NC))],
                ins=[bnF[1][:, :].opt()], outs=[tabF[1][0:TABROWS, :].opt()])

            # ---- label convs 1..10 interleaved with feature conv 2 ----
            def make_label_cb(j):
                last = (j == NUM_LBL)

                def cb(w, acc):
                    if last:
                        dst = xlfin[:, w * OUT:(w + 1) * OUT]
                        nc.vector.tensor_scalar(out=dst, in0=acc[:],
                                                scalar1=dinvb[:, w:w + 1],
                                                scalar2=None,
                                                op0=mybir.AluOpType.mult)
                        nc.vector.copy_predicated(
                            out=dst, mask=maskb[:, w:w + 1].to_broadcast([P, OUT]),
                            data=yb[:, w * OUT:(w + 1) * OUT])
                    else:
                        dst = vL[:, w * OUT:(w + 1) * OUT]
                        nc.vector.tensor_scalar(out=dst, in0=acc[:],
                                                scalar1=recipb[:, w:w + 1],
                                                scalar2=None,
                                                op0=mybir.AluOpType.mult)
                        nc.vector.copy_predicated(
                            out=dst, mask=maskb[:, w:w + 1].to_broadcast([P, OUT]),
                            data=dinvy[:, w * OUT:(w + 1) * OUT])
                return cb

            def cb_f2(w, acc):
                dst = hfin[:, w * HID:(w + 1) * HID]
                nc.vector.tensor_scalar(out=dst, in0=acc[:],
                                        scalar1=dinvb[:, w:w + 1], scalar2=None,
                                        op0=mybir.AluOpType.mult)
                if bias1 is not None:
                    nc.vector.tensor_add(out=dst, in0=dst, in1=bias1[:])

            for j in range(1, NUM_LBL + 1):
                stage_agg(tabL[(j - 1) % 2], OUT, bnL[(j - 1) % 2],
                          make_label_cb(j))
                if j == 1:
                    stage_agg(tabF[1], HID, bnF[1], cb_f2)  # feature conv 2
                if j < NUM_LBL:
                    stage_matmul(vL, OUT, Wl[:, j * OUT:(j + 1) * OUT], OUT,
                                 bnL[j % 2])
                    nc.gpsimd.collective_compute(
                        "AllGather", bass.mybir.AluOpType.bypass,
                        replica_groups=[list(range(NC))],
                        ins=[bnL[j % 2][:, :].opt()],
                        outs=[tabL[j % 2][0:TABROWS, :].opt()])

            # ---- fuse: sigmoid([h | xl | dw] @ Wf + bf) ----
            for w in range(NWIN):
                dwt = wk.tile([P, DW], f32, tag="dwt")
                nc.sync.dma_start(out=dwt[:], in_=dw_sh[w * P:(w + 1) * P, :])
                fTa = wk.tile([P, P], f32, tag="fTa")
                fTb = wk.tile([DW - 32, P], f32, tag="fTb")
                tp = ps.tile([P, P], f32, tag="tps")
                nc.tensor.transpose(out=tp[0:HID, :],
                                    in_=hfin[:, w * HID:(w + 1) * HID],
                                    identity=ident[:])
                nc.scalar.copy(out=fTa[0:HID, :], in_=tp[0:HID, :])
                tp2 = ps.tile([P, P], f32, tag="tps")
                nc.tensor.transpose(out=tp2[0:OUT, :],
                                    in_=xlfin[:, w * OUT:(w + 1) * OUT],
                                    identity=ident[:])
                nc.scalar.copy(out=fTa[HID:HID + OUT, :], in_=tp2[0:OUT, :])
                tp3 = ps.tile([P, P], f32, tag="tps")
                nc.tensor.transpose(out=tp3[0:DW, :], in_=dwt[:],
                                    identity=ident[:])
                nc.scalar.copy(out=fTa[HID + OUT:P, :],
                               in_=tp3[0:P - HID - OUT, :])
                nc.scalar.copy(out=fTb[:, :], in_=tp3[P - HID - OUT:DW, :])
                op = ps.tile([P, OUT], f32, tag="ops")
                nc.tensor.matmul(out=op[:], lhsT=fTa[:], rhs=Wfa[:],
                                 start=True, stop=False)
                nc.tensor.matmul(out=op[:], lhsT=fTb[:], rhs=Wfb[:],
                                 start=False, stop=True)
                ot = wk.tile([P, OUT], f32, tag="ot")
                if biasf is not None:
                    nc.vector.tensor_add(out=op[:], in0=op[:], in1=biasf[:])
                nc.scalar.activation(out=ot[:], in_=op[:],
                                     func=bass.mybir.ActivationFunctionType.Sigmoid)
                nc.sync.dma_start(out=out_sh[w * P:(w + 1) * P, :], in_=ot[:])

    nc.compile()
    return nc


_CACHE = {}


def kernel(x, y, edge_index, deep_walk_emb, label_input_mask,
           W_gcn0, b_gcn0, W_gcn1, b_gcn1, W_label, b_label, W_fuse, b_fuse):
    import concourse.bass_utils as bass_utils

    n_nodes = x.shape[0]
    ei = np.asarray(edge_index, dtype=np.int64)
    idx_all, meta = _preprocess(ei, n_nodes)
    core_of, local_of = meta["core_of"], meta["local_of"]

    nonzero_b = (bool(np.any(np.asarray(b_gcn0))),
                 bool(np.any(np.asarray(b_gcn1))),
                 bool(np.any(np.asarray(b_label))),
                 bool(np.any(np.asarray(b_fuse))))
    if nonzero_b[2]:
        raise NotImplementedError("nonzero label bias not wired")

    key = ("k1", n_nodes, ei.shape[1], nonzero_b)
    if key not in _CACHE:
        _CACHE[key] = _build(meta, nonzero_b)
    nc = _CACHE[key]

    x_s = _shard_nodes(x, core_of, local_of, IN_DIM)
    y_s = _shard_nodes(y, core_of, local_of, OUT)
    dw_s = _shard_nodes(deep_walk_emb, core_of, local_of, DW)
    mk_s = _shard_nodes(np.asarray(label_input_mask, np.int8)[:, None],
                       core_of, local_of, 1, dtype=np.int8)
    dg_s = np.zeros((NC, NPAD, 1), np.int32)
    dg_s[core_of, local_of, 0] = meta["deg"].astype(np.int32)

    bmax = max(HID, OUT) * NUM_LBL
    b_all = np.zeros((4, bmax), np.float32)
    b_all[0, :HID] = np.asarray(b_gcn0, np.float32)
    b_all[1, :HID] = np.asarray(b_gcn1, np.float32)
    b_all[2, :OUT * NUM_LBL] = np.asarray(b_label, np.float32).reshape(-1)
    b_all[3, :OUT] = np.asarray(b_fuse, np.float32)

    Wl_flat = np.asarray(W_label, np.float32).reshape(NUM_LBL * OUT, OUT)
    idx128 = np.tile(idx_all, (1, 8, 1))   # replicate 16-part wrap to 128

    in_maps = []
    for c in range(NC):
        in_maps.append({
            "x_sh": x_s[c], "y_sh": y_s[c], "dw_sh": dw_s[c],
            "mask_sh": mk_s[c], "deg_sh": dg_s[c],
            "idx_d": idx128[c],
            "W0": np.asarray(W_gcn0, np.float32),
            "W1": np.asarray(W_gcn1, np.float32),
            "Wl": Wl_flat,
            "Wf": np.asarray(W_fuse, np.float32),
            "b_all": b_all,
        })
    res = bass_utils.run_bass_kernel_spmd(nc, in_maps, core_ids=list(range(NC)))
    out = np.empty((n_nodes, OUT), np.float32)
    for c in range(NC):
        sel = core_of == np.int64(c)
        out[sel] = res.results[c]["out_sh"][local_of[sel]]
    return out


# revision 16
# speedup vs baseline: 1.5605x; 1.5605x over previous
"""FPLPGCN (2x GCNConv feature prop + 10x label prop + fuse) on 8 trn2 cores.

Strategy (graph/data parallel, per sharding hint):
- Nodes are globally sorted by in-degree and striped round-robin across the 8
  cores (degree-matched windows).  Each core owns NPAD=12544 node rows (44
  zero "fake" rows pad 100000 -> 100352).
- GCN math is refactored so aggregation is a pure gather+sum:
      out[n] = dinv[n] * (sum_{e->n} u'[src_e] + u'[n]) + b,
  with u' = dinv * (z @ W) the "table" that is AllGather'ed each round.
- Per conv round: each core computes u' for its nodes (PE matmul), AllGathers
  the table to HBM, then aggregates its in-edges with bulk dma_gather calls
  (node-aligned slot grids, int16 indices, 4 base-offset chunks to cover the
  100k-row table) and DVE strided reduces.
"""

import sys

sys.path.insert(0, "/opt/trn_rl_repo")

import numpy as np

NC = 8
P = 128
NPAD = 12544          # local rows per core (98 windows of 128)
NWIN = NPAD // P      # 98
TABROWS = NC * NPAD   # 100352
CH = TABROWS // 4     # 25088 chunk stride
WINR = 32768          # rows reachable per gather call (int16 >= 0)
SPAN = 3              # windows per gather-call group
IN_DIM, HID, OUT, DW = 128, 64, 32, 64
NUM_LBL = 10
FSLOT = 64            # gather element = 64 f32 = 256B (labels pad 32->64)


# ----------------------------------------------------------------------------
# host-side index preprocessing (pure index manipulation; no FP math on data)
# ----------------------------------------------------------------------------

def _preprocess(edge_index, n_nodes):
    src = np.asarray(edge_index[0], dtype=np.int64)
    dst = np.asarray(edge_index[1], dtype=np.int64)
    E = src.shape[0]
    deg = np.bincount(dst, minlength=n_nodes).astype(np.int64)

    order = np.argsort(deg, kind="stable")          # ascending in-degree
    rank = np.empty(n_nodes, np.int64)
    rank[order] = np.arange(TABROWS - n_nodes, TABROWS)  # fakes get ranks 0..351
    core_of = rank % NC
    local_of = rank // NC
    trow = core_of * NPAD + local_of                # table row per real node

    # per-edge placement
    ecore = core_of[dst]
    eloc = local_of[dst]
    ewin = eloc // P
    epart = eloc % P
    etr_s = trow[src]
    home = etr_s // CH
    flex = ((etr_s % CH) < (WINR - CH)) & (home >= 1)

    # node-chunk slot counts with greedy flex balancing (chunk h -> h-1)
    nodekey = ecore * NPAD + eloc                   # 0..TABROWS-1
    cnt = np.zeros((TABROWS, 4), np.int32)
    np.add.at(cnt, (nodekey[~flex], home[~flex]), 1)
    echunk = home.copy()
    for h in (1, 2, 3):
        m = flex & (home == h)
        if not m.any():
            continue
        f = np.bincount(nodekey[m], minlength=TABROWS)
        lo, hi = cnt[:, h - 1], cnt[:, h]
        x = np.clip((hi + f - lo + 1) // 2, 0, f)   # how many go to lower chunk
        # choose the first x flex edges of each node (stable order)
        o = np.argsort(nodekey[m], kind="stable")
        idxs = np.flatnonzero(m)[o]
        nk = nodekey[idxs]
        starts = np.searchsorted(nk, np.arange(TABROWS))
        pos = np.arange(idxs.size) - starts[nk]
        tolow = pos < x[nk]
        echunk[idxs[tolow]] = h - 1
        np.add.at(cnt, (nk, np.where(tolow, h - 1, h)), 1)

    # common column counts per (window, chunk) across all cores
    K = cnt.reshape(NC, NWIN, P, 4).max(axis=(0, 2))          # [NWIN, 4]

    # k-index of each edge within its (node, chunk) group
    o = np.lexsort((echunk, nodekey))
    nk = nodekey[o]
    ck = echunk[o]
    key = nk * 4 + ck
    first = np.searchsorted(key, key, side="left")
    kidx = np.empty(E, np.int64)
    kidx[o] = np.arange(E) - first

    # span/call layout: spans of SPAN windows; call = (span, chunk)
    nspan = (NWIN + SPAN - 1) // SPAN
    span_of_w = np.arange(NWIN) // SPAN
    # column offset of (w, c) inside call (span(w), c)
    colbase = np.zeros((NWIN, 4), np.int64)
    for s in range(nspan):
        ws = np.arange(s * SPAN, min((s + 1) * SPAN, NWIN))
        for c in range(4):
            acc = 0
            for w in ws:
                colbase[w, c] = acc
                acc += K[w, c]
    call_cols = np.zeros((nspan, 4), np.int64)
    for s in range(nspan):
        ws = slice(s * SPAN, min((s + 1) * SPAN, NWIN))
        call_cols[s] = K[ws].sum(axis=0)
    call_n = call_cols * P                                    # num_idxs per call
    # idx dram layout: calls concatenated (span-major, chunk-minor), each call
    # is n/16 int16 columns wrapped into 16 partitions, replicated x8.
    call_off16 = np.zeros((nspan, 4), np.int64)
    off = 0
    for s in range(nspan):
        for c in range(4):
            call_off16[s, c] = off
            off += call_n[s, c] // 16
    tot16 = off

    # fill idx buffers per core (default 0 -> chunk base row = fake zero row)
    idx_all = np.zeros((NC, 16, tot16), np.int16)
    ci = call_off16[span_of_w[ewin], echunk]                  # per-edge call off
    col = colbase[ewin, echunk] + kidx
    i_in_call = col * P + epart
    rel = etr_s - echunk * CH
    assert rel.min() >= 0 and rel.max() < WINR
    flat = ci * 16 + (i_in_call // 16) * 16 + (i_in_call % 16)
    # layout: [16, tot16]: position i of call -> [i%16, off16 + i//16]
    r16 = i_in_call % 16
    c16 = ci + i_in_call // 16
    idx_all[ecore, r16, c16] = rel.astype(np.int16)

    meta = dict(order=order, rank=rank, core_of=core_of, local_of=local_of,
                trow=trow, deg=deg, K=K, call_cols=call_cols, call_n=call_n,
                call_off16=call_off16, tot16=tot16, nspan=nspan)
    return idx_all, meta


def _shard_nodes(arr, core_of, local_of, width, dtype=np.float32):
    """Scatter full [N, width] node array into per-core [NPAD, width] shards."""
    n = arr.shape[0]
    out = np.zeros((NC, NPAD, width), dtype)
    a2 = np.asarray(arr, dtype).reshape(n, width)
    out[core_of, local_of] = a2
    return out


# ----------------------------------------------------------------------------
# device program
# ----------------------------------------------------------------------------

def _build(meta, nonzero_b):
    import concourse.bacc as bacc
    import concourse.bass as bass
    import concourse.mybir as mybir
    import concourse.tile as tile

    f32 = mybir.dt.float32
    K = meta["K"]
    call_n = meta["call_n"]
    call_off16 = meta["call_off16"]
    tot16 = meta["tot16"]
    nspan = meta["nspan"]
    TAB_ALLOC = 3 * CH + WINR  # rows needed so table[3*CH : 3*CH+WINR] is valid

    nc = bacc.Bacc("TRN2", target_bir_lowering=False, debug=False,
                   num_devices=NC, num_swdge_queues=4)

    x_sh = nc.dram_tensor("x_sh", [NPAD, IN_DIM], f32, kind="ExternalInput")
    y_sh = nc.dram_tensor("y_sh", [NPAD, OUT], f32, kind="ExternalInput")
    dw_sh = nc.dram_tensor("dw_sh", [NPAD, DW], f32, kind="ExternalInput")
    mask_sh = nc.dram_tensor("mask_sh", [NPAD, 1], mybir.dt.int8,
                             kind="ExternalInput")
    deg_sh = nc.dram_tensor("deg_sh", [NPAD, 1], mybir.dt.int32,
                            kind="ExternalInput")
    idx_d = nc.dram_tensor("idx_d", [P, tot16], mybir.dt.int16,
                           kind="ExternalInput")
    W0_d = nc.dram_tensor("W0", [IN_DIM, HID], f32, kind="ExternalInput")
    W1_d = nc.dram_tensor("W1", [HID, HID], f32, kind="ExternalInput")
    Wl_d = nc.dram_tensor("Wl", [NUM_LBL * OUT, OUT], f32, kind="ExternalInput")
    Wf_d = nc.dram_tensor("Wf", [HID + OUT + DW, OUT], f32, kind="ExternalInput")
    b_d = nc.dram_tensor("b_all", [4, max(HID, OUT) * NUM_LBL], f32,
                         kind="ExternalInput")  # b0|b1|bl(10x32)|bf rows
    out_sh = nc.dram_tensor("out_sh", [NPAD, OUT], f32, kind="ExternalOutput")

    # internal DRAM
    tabF = [nc.dram_tensor(f"tabF{i}", [TAB_ALLOC, FSLOT], f32,
                           addr_space="Shared") for i in range(2)]
    tabL = [nc.dram_tensor(f"tabL{i}", [TAB_ALLOC, FSLOT], f32,
                           addr_space="Shared") for i in range(2)]
    bnF = [nc.dram_tensor(f"bnF{i}", [NPAD, FSLOT], f32) for i in range(2)]
    bnL = [nc.dram_tensor(f"bnL{i}", [NPAD, FSLOT], f32) for i in range(2)]

    with tile.TileContext(nc) as tc:
        with tc.tile_pool(name="persist", bufs=1) as pp, \
             tc.tile_pool(name="g", bufs=2) as gp, \
             tc.tile_pool(name="ix", bufs=2) as ixp, \
             tc.tile_pool(name="wk", bufs=3) as wk, \
             tc.tile_pool(name="ps", bufs=2, space="PSUM") as ps, \
             tc.tile_pool(name="ub", bufs=2) as ub:

            # ---- constants / persistent state ----
            W0 = pp.tile([IN_DIM, HID], f32); nc.sync.dma_start(out=W0[:], in_=W0_d[:, :])
            # W1 / Wl replicated along partition offsets so batched-transpose
            # lhsT slices (base partition 64a / 32a) see rhs at the same base.
            W1 = pp.tile([P, HID], f32)
            for a in range(P // HID):
                nc.sync.dma_start(out=W1[a * HID:(a + 1) * HID, :], in_=W1_d[:, :])
            Wl = pp.tile([P, NUM_LBL * OUT], f32)
            for j in range(NUM_LBL):
                for a in range(P // OUT):
                    nc.sync.dma_start(
                        out=Wl[a * OUT:(a + 1) * OUT, j * OUT:(j + 1) * OUT],
                        in_=Wl_d[j * OUT:(j + 1) * OUT, :])
            Wfa = pp.tile([128, OUT], f32); nc.sync.dma_start(out=Wfa[:], in_=Wf_d[0:128, :])
            Wfb = pp.tile([HID + OUT + DW - 128, OUT], f32)
            nc.sync.dma_start(out=Wfb[:], in_=Wf_d[128:, :])
            from concourse.masks import make_identity
            ident = pp.tile([P, P], f32)
            make_identity(nc, ident[:])

            yb = pp.tile([P, NWIN * OUT], f32)
            nc.sync.dma_start(
                out=yb[:].rearrange("p (w f) -> p w f", w=NWIN),
                in_=y_sh[:, :].rearrange("(w p) f -> p w f", p=P))
            maskb = pp.tile([P, NWIN], mybir.dt.int8)
            nc.sync.dma_start(
                out=maskb[:],
                in_=mask_sh[:, 0].rearrange("(w p) -> p w", p=P))
            degb = pp.tile([P, NWIN], mybir.dt.int32)
            nc.sync.dma_start(
                out=degb[:],
                in_=deg_sh[:, 0].rearrange("(w p) -> p w", p=P))

            degf = pp.tile([P, NWIN], f32)
            nc.vector.tensor_copy(out=degf[:], in_=degb[:])
            recipb = pp.tile([P, NWIN], f32)
            nc.vector.tensor_scalar(out=degf[:], in0=degf[:], scalar1=1.0,
                                    scalar2=None, op0=mybir.AluOpType.add)
            nc.vector.reciprocal(out=recipb[:], in_=degf[:])      # 1/(deg+1)
            dinvb = pp.tile([P, NWIN], f32)
            nc.scalar.sqrt(out=dinvb[:], in_=recipb[:])           # 1/sqrt(deg+1)
            # zero fake lanes (window 0, partitions 0..43)
            nc.vector.memset(recipb[0:44, 0:1], 0.0)
            nc.vector.memset(dinvb[0:44, 0:1], 0.0)
            dinvy = pp.tile([P, NWIN * OUT], f32)
            for w in range(NWIN):
                nc.vector.tensor_scalar(
                    out=dinvy[:, w * OUT:(w + 1) * OUT],
                    in0=yb[:, w * OUT:(w + 1) * OUT],
                    scalar1=dinvb[:, w:w + 1], scalar2=None,
                    op0=mybir.AluOpType.mult)

            # bias broadcast tiles (built only when biases are nonzero)
            def bias_tile(row, width):
                bt = pp.tile([P, width], f32, tag=f"bias{row}_{width}", name=f"bias{row}_{width}")
                onecol = pp.tile([1, P], f32, tag="onecol", name="onecol")
                nc.vector.memset(onecol[:], 1.0)
                brow = pp.tile([1, width], f32, tag=f"brow{row}_{width}", name=f"brow{row}_{width}")
                nc.sync.dma_start(out=brow[:], in_=b_d[row:row + 1, 0:width])
                pt = ps.tile([P, width], f32, tag="biasps", name="biasps")
                nc.tensor.matmul(out=pt[:], lhsT=onecol[:], rhs=brow[:],
                                 start=True, stop=True)
                nc.vector.tensor_copy(out=bt[:], in_=pt[:])
                return bt

            bias0 = bias_tile(0, HID) if nonzero_b[0] else None
            bias1 = bias_tile(1, HID) if nonzero_b[1] else None
            biasf = bias_tile(3, OUT) if nonzero_b[3] else None
            # label biases: one [P, OUT] tile per label conv if any nonzero
            biasl = [None] * NUM_LBL

            vF = pp.tile([P, NWIN * HID], f32)      # current v (feature chain)
            vL = pp.tile([P, NWIN * OUT], f32)      # current v (label chain)
            hfin = pp.tile([P, NWIN * HID], f32)    # final h (node major)
            xlfin = pp.tile([P, NWIN * OUT], f32)   # final xl (node major)

            # ---- helpers ----
            def stage_matmul(vtile, F_in, W_ap, F_out, bounce):
                """u' = v @ W per window -> bounce DRAM."""
                per = min(P // F_in, 3)   # lhsT base partition must be 0/32/64
                for w0 in range(0, NWIN, per):
                    nwt = min(per, NWIN - w0)
                    tp = ps.tile([P, P], f32, tag="tps")
                    nc.tensor.transpose(
                        out=tp[0:nwt * F_in, :],
                        in_=vtile[:, w0 * F_in:(w0 + nwt) * F_in],
                        identity=ident[:])
                    vT = wk.tile([P, P], f32, tag="vT")
                    nc.scalar.copy(out=vT[0:nwt * F_in, :], in_=tp[0:nwt * F_in, :])
                    for a in range(nwt):
                        w = w0 + a
                        up = ps.tile([P, F_out], f32, tag="ups")
                        nc.tensor.matmul(out=up[:],
                                         lhsT=vT[a * F_in:(a + 1) * F_in, :],
                                         rhs=W_ap[a * F_in:(a + 1) * F_in, :],
                                         start=True, stop=True)
                        ut = wk.tile([P, F_out], f32, tag="ut")
                        nc.scalar.copy(out=ut[:], in_=up[:])
                        nc.sync.dma_start(
                            out=bounce[w * P:(w + 1) * P, 0:F_out], in_=ut[:])

            def stage_agg(tab, F_out, bounce, out_cb):
                """Aggregate: out_cb(w, acc_tile) for each window."""
                qctr = [0]
                for s in range(nspan):
                    w_lo = s * SPAN
                    w_hi = min(w_lo + SPAN, NWIN)
                    ncol16 = int(sum(call_n[s, c] // 16 for c in range(4)))
                    if ncol16 == 0:
                        continue
                    ixt = ixp.tile([P, ncol16], mybir.dt.int16, tag="ix")
                    base16 = int(call_off16[s, 0])
                    nc.sync.dma_start(out=ixt[:],
                                      in_=idx_d[:, base16:base16 + ncol16])
                    span_cols = int(sum(K[w_lo:w_hi, c].sum() for c in range(4)))
                    g = gp.tile([P, span_cols * FSLOT], f32, tag="g")
                    coff = 0
                    reg = {}
                    SUBCOLS = 8       # <=1024 idxs per dma_gather call
                    for c in range(4):
                        n = int(call_n[s, c])
                        if n == 0:
                            continue
                        o16 = int(call_off16[s, c]) - base16
                        ncols = n // P
                        for c0 in range(0, ncols, SUBCOLS):
                            c1 = min(c0 + SUBCOLS, ncols)
                            nsub = (c1 - c0) * P
                            nc.gpsimd.dma_gather(
                                out_ap=g[:, (coff + c0) * FSLOT:(coff + c1) * FSLOT]
                                    .rearrange("p (s f) -> p s f", f=FSLOT),
                                in_ap=tab[c * CH:c * CH + WINR, :],
                                idxs_ap=ixt[:, o16 + c0 * 8:o16 + c1 * 8],
                                num_idxs=nsub, num_idxs_reg=nsub,
                                elem_size=FSLOT, queue_num=qctr[0] % 4)
                            qctr[0] += 1
                        # record region columns per window
                        cc = coff
                        for w in range(w_lo, w_hi):
                            if K[w, c]:
                                reg.setdefault(w, []).append((cc, int(K[w, c])))
                                cc += int(K[w, c])
                        coff += n // P
                    for w in range(w_lo, w_hi):
                        acc = wk.tile([P, F_out], f32, tag="acc")
                        selft = wk.tile([P, F_out], f32, tag="selft")
                        nc.sync.dma_start(out=selft[:],
                                          in_=bounce[w * P:(w + 1) * P, 0:F_out])
                        cur = selft[:]
                        first = True
                        for (cstart, ncols) in reg.get(w, []):
                            gv = g[:, cstart * FSLOT:(cstart + ncols) * FSLOT] \
                                .rearrange("p (k f) -> p f k", f=FSLOT)
                            if first:
                                nc.vector.reduce_sum(
                                    out=acc[:], in_=gv[:, 0:F_out, :],
                                    axis=mybir.AxisListType.X)
                                first = False
                            else:
                                t = wk.tile([P, F_out], f32, tag="rt")
                                nc.vector.reduce_sum(
                                    out=t[:], in_=gv[:, 0:F_out, :],
                                    axis=mybir.AxisListType.X)
                                nc.vector.tensor_add(out=acc[:], in0=acc[:],
                                                     in1=t[:])
                        if first:
                            nc.vector.tensor_copy(out=acc[:], in_=cur)
                        else:
                            nc.vector.tensor_add(out=acc[:], in0=acc[:],
                                                 in1=cur)
                        out_cb(w, acc)

            # ---- feature conv 1: v_x = dinv*x ; u_f1 = v_x @ W0 ----
            for w in range(NWIN):
                xt = wk.tile([P, IN_DIM], f32, tag="xt")
                nc.sync.dma_start(out=xt[:], in_=x_sh[w * P:(w + 1) * P, :])
                nc.vector.tensor_scalar(out=xt[:], in0=xt[:],
                                        scalar1=dinvb[:, w:w + 1],
                                        scalar2=None, op0=mybir.AluOpType.mult)
                # transpose+matmul inline (F_in=128: one window per transpose)
                tp = ps.tile([P, P], f32, tag="tps")
                nc.tensor.transpose(out=tp[:], in_=xt[:], identity=ident[:])
                vT = wk.tile([P, P], f32, tag="vT")
                nc.scalar.copy(out=vT[:], in_=tp[:])
                up = ps.tile([P, HID], f32, tag="ups")
                nc.tensor.matmul(out=up[:], lhsT=vT[:], rhs=W0[:], start=True,
                                 stop=True)
                ut = wk.tile([P, HID], f32, tag="ut")
                nc.scalar.copy(out=ut[:], in_=up[:])
                nc.sync.dma_start(out=bnF[0][w * P:(w + 1) * P, 0:HID], in_=ut[:])
            nc.gpsimd.collective_compute(
                "AllGather", bass.mybir.AluOpType.bypass,
                replica_groups=[list(range(NC))],
                ins=[bnF[0][:, :].opt()], outs=[tabF[0][0:TABROWS, :].opt()])

            # ---- label conv 1 input: u_l1 = dinvy @ Wl0 ----
            stage_matmul(dinvy, OUT, Wl[:, 0:OUT], OUT, bnL[0])
            nc.gpsimd.collective_compute(
                "AllGather", bass.mybir.AluOpType.bypass,
                replica_groups=[list(range(NC))],
                ins=[bnL[0][:, :].opt()], outs=[tabL[0][0:TABROWS, :].opt()])

            # ---- feature conv 1 aggregate -> v_f1 ; u_f2 ; AG ----
            def cb_f1(w, acc):
                nc.vector.tensor_scalar(out=vF[:, w * HID:(w + 1) * HID],
                                        in0=acc[:], scalar1=recipb[:, w:w + 1],
                                        scalar2=None, op0=mybir.AluOpType.mult)
                if bias0 is not None:
                    dv = wk.tile([P, HID], f32, tag="dbv")
                    nc.vector.tensor_scalar(out=dv[:], in0=bias0[:],
                                            scalar1=dinvb[:, w:w + 1],
                                            scalar2=None,
                                            op0=mybir.AluOpType.mult)
                    nc.vector.tensor_add(out=vF[:, w * HID:(w + 1) * HID],
                                         in0=vF[:, w * HID:(w + 1) * HID],
                                         in1=dv[:])
            stage_agg(tabF[0], HID, bnF[0], cb_f1)
            stage_matmul(vF, HID, W1[:, :], HID, bnF[1])
            nc.gpsimd.collective_compute(
                "AllGather", bass.mybir.AluOpType.bypass,
                replica_groups=[list(range(NC))],
                ins=[bnF[1][:, :].opt()], outs=[tabF[1][0:TABROWS, :].opt()])

            # ---- label convs 1..10 interleaved with feature conv 2 ----
            def make_label_cb(j):
                last = (j == NUM_LBL)

                def cb(w, acc):
                    if last:
                        dst = xlfin[:, w * OUT:(w + 1) * OUT]
                        nc.vector.tensor_scalar(out=dst, in0=acc[:],
                                                scalar1=dinvb[:, w:w + 1],
                                                scalar2=None,
                                                op0=mybir.AluOpType.mult)
                        nc.vector.copy_predicated(
                            out=dst, mask=maskb[:, w:w + 1].to_broadcast([P, OUT]),
                            data=yb[:, w * OUT:(w + 1) * OUT])
                    else:
                        dst = vL[:, w * OUT:(w + 1) * OUT]
                        nc.vector.tensor_scalar(out=dst, in0=acc[:],
                                                scalar1=recipb[:, w:w + 1],
                                                scalar2=None,
                                                op0=mybir.AluOpType.mult)
                        nc.vector.copy_predicated(
                            out=dst, mask=maskb[:, w:w + 1].to_broadcast([P, OUT]),
                            data=dinvy[:, w * OUT:(w + 1) * OUT])
                return cb

            def cb_f2(w, acc):
                dst = hfin[:, w * HID:(w + 1) * HID]
                nc.vector.tensor_scalar(out=dst, in0=acc[:],
                                        scalar1=dinvb[:, w:w + 1], scalar2=None,
                                        op0=mybir.AluOpType.mult)
                if bias1 is not None:
                    nc.vector.tensor_add(out=dst, in0=dst, in1=bias1[:])

            for j in range(1, NUM_LBL + 1):
                stage_agg(tabL[(j - 1) % 2], OUT, bnL[(j - 1) % 2],
                          make_label_cb(j))
                if j == 1:
                    stage_agg(tabF[1], HID, bnF[1], cb_f2)  # feature conv 2
                if j < NUM_LBL:
                    stage_matmul(vL, OUT, Wl[:, j * OUT:(j + 1) * OUT], OUT,
                                 bnL[j % 2])
                    nc.gpsimd.collective_compute(
                        "AllGather", bass.mybir.AluOpType.bypass,
                        replica_groups=[list(range(NC))],
                        ins=[bnL[j % 2][:, :].opt()],
                        outs=[tabL[j % 2][0:TABROWS, :].opt()])

            # ---- fuse: sigmoid([h | xl | dw] @ Wf + bf) ----
            for w in range(NWIN):
                dwt = wk.tile([P, DW], f32, tag="dwt")
                nc.sync.dma_start(out=dwt[:], in_=dw_sh[w * P:(w + 1) * P, :])
                fTa = wk.tile([P, P], f32, tag="fTa")
                fTb = wk.tile([DW - 32, P], f32, tag="fTb")
                tp = ps.tile([P, P], f32, tag="tps")
                nc.tensor.transpose(out=tp[0:HID, :],
                                    in_=hfin[:, w * HID:(w + 1) * HID],
                                    identity=ident[:])
                nc.scalar.copy(out=fTa[0:HID, :], in_=tp[0:HID, :])
                tp2 = ps.tile([P, P], f32, tag="tps")
                nc.tensor.transpose(out=tp2[0:OUT, :],
                                    in_=xlfin[:, w * OUT:(w + 1) * OUT],
                                    identity=ident[:])
                nc.scalar.copy(out=fTa[HID:HID + OUT, :], in_=tp2[0:OUT, :])
                tp3 = ps.tile([P, P], f32, tag="tps")
                nc.tensor.transpose(out=tp3[0:DW, :], in_=dwt[:],
                                    identity=ident[:])
                nc.scalar.copy(out=fTa[HID + OUT:P, :],
                               in_=tp3[0:P - HID - OUT, :])
                nc.scalar.copy(out=fTb[:, :], in_=tp3[P - HID - OUT:DW, :])
                op = ps.tile([P, OUT], f32, tag="ops")
                nc.tensor.matmul(out=op[:], lhsT=fTa[:], rhs=Wfa[:],
                                 start=True, stop=False)
                nc.tensor.matmul(out=op[:], lhsT=fTb[:], rhs=Wfb[:],
                                 start=False, stop=True)
                ot = wk.tile([P, OUT], f32, tag="ot")
                if biasf is not None:
                    nc.vector.tensor_add(out=op[:], in0=op[:], in1=biasf[:])
                nc.scalar.activation(out=ot[:], in_=op[:],
                                     func=bass.mybir.ActivationFunctionType.Sigmoid)
                nc.sync.dma_start(out=out_sh[w * P:(w + 1) * P, :], in_=ot[:])

    nc.compile()
    return nc


_CACHE = {}


def kernel(x, y, edge_index, deep_walk_emb, label_input_mask,
           W_gcn0, b_gcn0, W_gcn1, b_gcn1, W_label, b_label, W_fuse, b_fuse):
    import concourse.bass_utils as bass_utils

    n_nodes = x.shape[0]
    ei = np.asarray(edge_index, dtype=np.int64)
    idx_all, meta = _preprocess(ei, n_nodes)
    core_of, local_of = meta["core_of"], meta["local_of"]

    nonzero_b = (bool(np.any(np.asarray(b_gcn0))),
                 bool(np.any(np.asarray(b_gcn1))),
                 bool(np.any(np.asarray(b_label))),
                 bool(np.any(np.asarray(b_fuse))))
    if nonzero_b[2]:
        raise NotImplementedError("nonzero label bias not wired")

    key = ("k1", n_nodes, ei.shape[1], nonzero_b)
    if key not in _CACHE:
        _CACHE[key] = _build(meta, nonzero_b)
    nc = _CACHE[key]

    x_s = _shard_nodes(x, core_of, local_of, IN_DIM)
    y_s = _shard_nodes(y, core_of, local_of, OUT)
    dw_s = _shard_nodes(deep_walk_emb, core_of, local_of, DW)
    mk_s = _shard_nodes(np.asarray(label_input_mask, np.int8)[:, None],
                       core_of, local_of, 1, dtype=np.int8)
    dg_s = np.zeros((NC, NPAD, 1), np.int32)
    dg_s[core_of, local_of, 0] = meta["deg"].astype(np.int32)

    bmax = max(HID, OUT) * NUM_LBL
    b_all = np.zeros((4, bmax), np.float32)
    b_all[0, :HID] = np.asarray(b_gcn0, np.float32)
    b_all[1, :HID] = np.asarray(b_gcn1, np.float32)
    b_all[2, :OUT * NUM_LBL] = np.asarray(b_label, np.float32).reshape(-1)
    b_all[3, :OUT] = np.asarray(b_fuse, np.float32)

    Wl_flat = np.asarray(W_label, np.float32).reshape(NUM_LBL * OUT, OUT)
    idx128 = np.tile(idx_all, (1, 8, 1))   # replicate 16-part wrap to 128

    in_maps = []
    for c in range(NC):
        in_maps.append({
            "x_sh": x_s[c], "y_sh": y_s[c], "dw_sh": dw_s[c],
            "mask_sh": mk_s[c], "deg_sh": dg_s[c],
            "idx_d": idx128[c],
            "W0": np.asarray(W_gcn0, np.float32),
            "W1": np.asarray(W_gcn1, np.float32),
            "Wl": Wl_flat,
            "Wf": np.asarray(W_fuse, np.float32),
            "b_all": b_all,
        })
    res = bass_utils.run_bass_kernel_spmd(nc, in_maps, core_ids=list(range(NC)))
    out = np.empty((n_nodes, OUT), np.float32)
    for c in range(NC):
        sel = core_of == np.int64(c)
        out[sel] = res.results[c]["out_sh"][local_of[sel]]
    return out
